# revision 56
# baseline (speedup 1.0000x reference)
"""Trainium2 Bass kernel for nn_Decoder (conductor-LSTM -> decoder-LSTM -> logits).

Sharding: pure data-parallel over batch B=256 -> 32 per core on 8 NeuronCores.
No collectives; each core runs an identical program on its batch slice.

On-chip layouts are "transposed": feature dims live on SBUF partitions, the
(s, b) row index lives on the free dim, so decoder matmuls are weight-stationary
[128,128] x [128,512] with fp32 PSUM accumulation.

Precision: the decoder scan (85% of FLOPs) and the pre-decoder (pre + ctx)
matmuls run in fp8e4 DoubleRow mode (2 contraction elements per PE cell,
~1.4-1.8x bf16 throughput): weights pre-quantized e4m3 at scale 2048,
h / dec_in / centered tokens at scale 64; the 2^-17 product scale is folded
into the gate-assembly op or the psum-reading activation. Teacher-forcing
tokens are centered (tok - 0.5) with 0.5*rowsum(W_tok) folded into pre_ctx,
which halves fp8 quantization noise; step 0 (zero token) undoes that fold
exactly via the gate activation bias. The logits matmul and the conductor
stay bf16 (their error hits the output directly / compounds); LSTM cell
state c and all PSUM accumulation stay fp32. Logits are written to HBM as
bf16 (host converts back to fp32) - the fp32 output drain cost ~72 us.
Measured rel err 1.659e-2 (emulator-exact; gate is 2e-2), HW 1.198 ms vs
the 2.100 ms bf16 baseline on the same slope methodology.

Scheduling: the decoder loop is software-pipelined over hidden-chunk groups
(chains+gate-assembly | activations | cell | h-writes lagged by 0/0/1/2
groups) so no strict-FIFO engine ever head-of-line blocks the PSUM-draining
ops that pace the PE; logits matmuls are k-major with the freshest h chunk
last. Elementwise ops are placed by measured HW cost (Act 0.45us < DVE
0.63us << Pool/GPSIMD 1.17us per [128,512] op; GPSIMD only runs plain
TensorTensor and cannot touch PSUM). The conductor folds its bias in as a
rank-1 ones x bias matmul, groups transposes per gate to cut 32 tiny
activations to 4, and runs a tiny dependent matmul mid-cell to keep the PE
HAM clock-gate from re-throttling during the serial cell gap.
"""

import os
import sys

for _p in ("/opt/trn_rl_repo", "/root/.axon_site/_ro/trn_rl_repo"):
    if os.path.isdir(_p) and _p not in sys.path:
        sys.path.insert(0, _p)

import ml_dtypes
import numpy as np

import concourse.bass as bass
import concourse.mybir as mybir
import concourse.tile as tile
from concourse import bacc
from concourse.bass import ts
from concourse.masks import make_identity

F32 = mybir.dt.float32
BF16 = mybir.dt.bfloat16
F8 = mybir.dt.float8e4
DR = mybir.MatmulPerfMode.DoubleRow
AF = mybir.ActivationFunctionType
ALU = mybir.AluOpType

W_SCALE = 2048.0  # e4m3 scale for decoder weights
H_SCALE = 64.0  # e4m3 scale for h and centered tokens
INV_PROD = 1.0 / (W_SCALE * H_SCALE)  # 2^-17, folded into gate assembly

B, T, Z, CH, LH, S, N = 256, 512, 512, 1024, 1024, 16, 16
NCORES = 8
BL = B // NCORES  # 32 batch rows per core


def _emit(nc, tc, prm, s_steps, n_steps, bl, px, a_reps=1):
    """Trace one full forward pass. px prefixes pool/tile names."""
    rows = s_steps * bl
    (p_zt, p_xt, p_wlin, p_blin, p_wcond, p_bcond, p_wpre, p_bpre, p_wctx,
     p_bdec, p_wtok, p_whh, p_wout, p_bout, p_tokneg, p_bpre64, p_out) = prm

    with (
        tc.tile_pool(name=f"{px}const", bufs=1) as pconst,
        tc.tile_pool(name=f"{px}state", bufs=1) as pstate,
    ):
        ident = pconst.tile([32, 32], F32, name=f"{px}ident")
        make_identity(nc, ident[:])
        blin = pconst.tile([128, 16], F32, name=f"{px}blin")
        nc.sync.dma_start(blin[:], p_blin[:])
        # conductor bias as a bf16 row: added into the gate PSUM via a
        # rank-1 (ones x bias) matmul appended to each wave chain
        bcond = pconst.tile([1, 4096], BF16, name=f"{px}bcond")
        nc.sync.dma_start(bcond[:], p_bcond[:])
        ones32 = pconst.tile([1, 32], BF16, name=f"{px}ones32")
        nc.vector.memset(ones32[:], 1.0)
        bpre = pconst.tile([128, 16], F32, name=f"{px}bpre")
        nc.sync.dma_start(bpre[:], p_bpre[:])
        bdec = pconst.tile([128, 32], F32, name=f"{px}bdec")
        nc.sync.dma_start(bdec[:], p_bdec[:])
        bout = pconst.tile([128, 4], F32, name=f"{px}bout")
        nc.sync.dma_start(bout[:], p_bout[:])
        tokneg = pconst.tile([128, 32], F32, name=f"{px}tokneg")
        nc.sync.dma_start(tokneg[:], p_tokneg[:])
        bpre64 = pconst.tile([128, 16], F32, name=f"{px}bpre64")
        nc.sync.dma_start(bpre64[:], p_bpre64[:])

        # Persistent decoder state. The fp8 h (scale 64, feeds the DoubleRow
        # gate matmuls) is double-buffered: within a step every hidden-chunk
        # matmul reads all 8 chunks of the old h, so the cell must not
        # overwrite them in place. h_bf is the bf16 h for the logits matmul.
        c_dec = pstate.tile([128, 8, rows], F32, name=f"{px}c_dec")
        h_bufs = [
            pstate.tile([128, 8, rows], F8, name=f"{px}h_dec{i}") for i in range(2)
        ]
        h_bf = pstate.tile([128, 8, rows], BF16, name=f"{px}h_bf")
        pre_ctx = pstate.tile([128, 32, rows], BF16, name=f"{px}pre_ctx")
        # decoder recurrent weights at top level: their DMA issues during
        # phase A (fp8 wcond freed the SBUF headroom), hiding the phase-C
        # entry bubble
        whh_sb = pstate.tile([128, 8, 4096], F8, name=f"{px}whh_sb")

        with tc.tile_pool(name=f"{px}ab", bufs=1) as pab:
            # conductor h history, laid out [p, k-chunk, s, b] so that
            # [:, k, s, :] is a [128, bl] matmul operand and [:, k] is a
            # contiguous [128, s*bl] rhs for the pre-decoder matmuls.
            # The bf16 copy drives the conductor recurrence; the e4m3*64
            # copy feeds phase B's fp8 DoubleRow pre-matmuls.
            h_all = pab.tile([128, 8, s_steps, bl], BF16, name=f"{px}h_all")
            h_all8 = pab.tile([128, 8, s_steps, bl], F8, name=f"{px}h_all8")

            # ---------------- phase A: linear_in + conductor scan ------
            with (
                tc.tile_pool(name=f"{px}aw", bufs=1) as paw,
                tc.tile_pool(name=f"{px}ag", bufs=8) as pgsb,
                tc.tile_pool(name=f"{px}acell", bufs=2) as pcell,
                tc.tile_pool(name=f"{px}aps", bufs=4, space="PSUM") as ppsa,
                tc.tile_pool(name=f"{px}atp", bufs=4, space="PSUM") as ptpa,
            ):
                wlin_sb = paw.tile([128, 4, 2048], BF16, name=f"{px}wlin_sb")
                for k in range(4):
                    nc.sync.dma_start(wlin_sb[:, k, :], p_wlin[k])
                # zt before the big wcond load so the lin_in chains can start
                # as soon as wlin+zt land (wcond is only needed ~8 matmuls in)
                zt_sb = paw.tile([128, 4, bl], BF16, name=f"{px}zt_sb")
                nc.sync.dma_start(zt_sb[:], p_zt[:])
                wcond_sb = paw.tile([128, 8, 4096], F8, name=f"{px}wcond_sb")
                for k in range(8):
                    nc.sync.dma_start(wcond_sb[:, k, :], p_wcond[k])
                for k in range(8):
                    nc.sync.dma_start(whh_sb[:, k, :], p_whh[k])

                h0 = paw.tile([128, 8, bl], BF16, name=f"{px}h0")
                c_cond = paw.tile([128, 8, bl], F32, name=f"{px}c_cond")

                # hc0_T = tanh(lin_in_w @ z_T + b): chunks 0-7 -> h0,
                # chunks 8-15 -> c0
                for m in range(16):
                    ps = ppsa.tile([128, bl], F32, tag="mm", name=f"{px}aps{m}")
                    for k in range(4):
                        nc.tensor.matmul(
                            ps[:],
                            wlin_sb[:, k, ts(m, 128)],
                            zt_sb[:, k, :],
                            start=(k == 0),
                            stop=(k == 3),
                        )
                    dst = h0[:, m, :] if m < 8 else c_cond[:, m - 8, :]
                    nc.scalar.activation(dst, ps[:], AF.Tanh, bias=blin[:, m : m + 1])

                # conductor: gates = h @ Whh.T + bias (input term is zero).
                # a_reps > 1 repeats the whole conductor pass (timing-only
                # builds, for differential phase-A measurement).
                for s in range(s_steps * a_reps):

                    def hsrc(k, s=s):
                        return (
                            h0[:, k, :]
                            if s == 0
                            else h_all[:, k, (s - 1) % s_steps, :]
                        )

                    si = pcell.tile([128, 8, bl], F32, tag="si", name=f"{px}si{s}")
                    sf = pcell.tile([128, 8, bl], F32, tag="sf", name=f"{px}sf{s}")
                    tg = pcell.tile([128, 8, bl], F32, tag="tg", name=f"{px}tg{s}")
                    so = pcell.tile([128, 8, bl], F32, tag="so", name=f"{px}so{s}")
                    # 2 waves of 4 column-group-tiled matmul chains: the four
                    # M=32 chains run concurrently in distinct 32-col strips
                    # of the PE array, packing the gates for 4 n-chunks into
                    # one [128, 512] psum tile. The conductor bias rides in as
                    # a rank-1 (ones x bias_row) matmul at the end of each
                    # chain, so the activations need no per-chunk bias.
                    for wave in range(2):
                        ps = ppsa.tile(
                            [128, 512], F32, tag="mm", name=f"{px}cps{s}_{wave}"
                        )
                        for k in range(8):
                            for j in range(4):
                                # skip_group_check: the sim's zero-region
                                # conflict guard is partition-base-blind; the
                                # four chains write disjoint 32-row slices.
                                nc.tensor.matmul(
                                    ps[ts(j, bl), :],
                                    hsrc(k),
                                    wcond_sb[:, k, ts(wave * 4 + j, 512)],
                                    start=(k == 0),
                                    stop=False,
                                    tile_position=(0, j * 32),
                                    skip_group_check=True,
                                )
                        for j in range(4):
                            nch = wave * 4 + j
                            nc.tensor.matmul(
                                ps[ts(j, bl), :],
                                ones32[:, :],
                                bcond[:, ts(nch, 512)],
                                start=False,
                                stop=True,
                                tile_position=(0, j * 32),
                                skip_group_check=True,
                            )
                        gtiles = []
                        for j in range(4):
                            nch = wave * 4 + j
                            g = pgsb.tile(
                                [bl, 512], F32, tag="g", name=f"{px}g{s}_{nch}"
                            )
                            nc.vector.tensor_copy(g[:], ps[ts(j, bl), :])
                            gtiles.append(g)
                        # transpose into per-gate [128, 8*32] staging, then a
                        # single activation per gate (gate g is complete
                        # within one wave: its 8 hc-chunks span two j strips)
                        for gl in range(2):
                            gate = wave * 2 + gl
                            tpg = ptpa.tile(
                                [128, 8, bl], F32, tag="tp", name=f"{px}tp{s}_{gate}"
                            )
                            for idx in range(8):
                                j = gl * 2 + idx // 4
                                jj = idx % 4
                                nc.tensor.transpose(
                                    tpg[:, idx, :],
                                    gtiles[j][:, ts(jj, 128)],
                                    ident[:],
                                )
                            dstt = (si, sf, tg, so)[gate]
                            fn = AF.Tanh if gate == 2 else AF.Sigmoid
                            nc.scalar.activation(
                                dstt[:, :, :], tpg[:], fn, scale=1.0 / W_SCALE
                            )
                    fc = pcell.tile([128, 8, bl], F32, tag="fc", name=f"{px}fc{s}")
                    nc.vector.scalar_tensor_tensor(
                        fc[:], sf[:], 0.0, c_cond[:], op0=ALU.bypass, op1=ALU.mult
                    )
                    ig = pcell.tile([128, 8, bl], F32, tag="ig", name=f"{px}ig{s}")
                    nc.vector.scalar_tensor_tensor(
                        ig[:], si[:], 0.0, tg[:], op0=ALU.bypass, op1=ALU.mult
                    )
                    nc.vector.tensor_add(c_cond[:], fc[:], ig[:])
                    tcc = pcell.tile([128, 8, bl], F32, tag="tcc", name=f"{px}tcc{s}")
                    nc.scalar.activation(tcc[:], c_cond[:], AF.Tanh)

                    nc.vector.scalar_tensor_tensor(
                        h_all[:, :, s % s_steps, :],
                        so[:],
                        0.0,
                        tcc[:],
                        op0=ALU.bypass,
                        op1=ALU.mult,
                    )
                    nc.vector.scalar_tensor_tensor(
                        h_all8[:, :, s % s_steps, :],
                        so[:],
                        H_SCALE,
                        tcc[:],
                        op0=ALU.mult,
                        op1=ALU.mult,
                    )
                    # HAM warm-keeper: a tiny matmul that depends on mid-gap
                    # data (the fresh h slice) splits the PE-idle window below
                    # the ~3.4us re-throttle threshold, keeping the conductor
                    # matmuls at 2.4 GHz
                    warm = ppsa.tile([32, 32], F32, tag="mm", name=f"{px}warm{s}")
                    nc.tensor.matmul(
                        warm[:],
                        ones32[:, :],
                        h_all[0:1, 0, s % s_steps, :],
                        start=True,
                        stop=True,
                    )

            # ---------------- phase B: pre-decoder -----------------------
            with (
                tc.tile_pool(name=f"{px}bw", bufs=1) as pbw,
                tc.tile_pool(name=f"{px}bctx", bufs=4) as pbctx,
                tc.tile_pool(name=f"{px}bps", bufs=8, space="PSUM") as ppsb,
            ):
                wpre_sb = pbw.tile([128, 8, 2048], F8, name=f"{px}wpre_sb")
                for k in range(8):
                    nc.sync.dma_start(wpre_sb[:, k, :], p_wpre[k])
                # dec_in quantized e4m3 at scale 64 (max|dec_in| ~0.44, so
                # 64x stays far below 240) for the fp8 DoubleRow ctx matmuls
                dec_in = pbw.tile([128, 16, rows], F8, name=f"{px}dec_in")

                # dec_in_T = pre_w @ cond_outs_T + pre_b (fp8 DoubleRow; the
                # PSUM carries 2^17 x the true value)
                for m in range(16):
                    ps = ppsb.tile([128, rows], F32, tag="ps", name=f"{px}bps{m}")
                    for kp in range(4):
                        nc.tensor.matmul(
                            ps[:],
                            wpre_sb[:, 2 * kp : 2 * kp + 2, ts(m, 128)],
                            h_all8[:, 2 * kp : 2 * kp + 2],
                            start=(kp == 0),
                            stop=(kp == 3),
                            perf_mode=DR,
                        )
                    nc.scalar.activation(
                        dec_in[:, m, :],
                        ps[:],
                        AF.Identity,
                        scale=H_SCALE * INV_PROD,
                        bias=bpre64[:, m : m + 1],
                    )
                    if m < 8:
                        # decoder h0, e4m3 at scale 64 (same value as dec_in)
                        nc.scalar.activation(
                            h_bufs[0][:, m, :],
                            ps[:],
                            AF.Identity,
                            scale=H_SCALE * INV_PROD,
                            bias=bpre64[:, m : m + 1],
                        )
                    else:
                        # decoder c0 (fp32): ps*2^-17 + pre_b
                        nc.vector.tensor_scalar(
                            c_dec[:, m - 8, :],
                            ps[:],
                            INV_PROD,
                            bpre[:, m : m + 1],
                            op0=ALU.mult,
                            op1=ALU.add,
                        )

                # pre_ctx_T = W_ctx @ dec_in_T + dec_bias (constant over n),
                # fp8 DoubleRow pairs over the 16 k-chunks
                for m in range(32):
                    wt = pbctx.tile(
                        [128, 16, 128], F8, tag="wctx", name=f"{px}wc{m}"
                    )
                    nc.sync.dma_start(wt[:], p_wctx[m])
                    ps = ppsb.tile([128, rows], F32, tag="ps", name=f"{px}xps{m}")
                    for kp in range(8):
                        nc.tensor.matmul(
                            ps[:],
                            wt[:, 2 * kp : 2 * kp + 2, :],
                            dec_in[:, 2 * kp : 2 * kp + 2, :],
                            start=(kp == 0),
                            stop=(kp == 7),
                            perf_mode=DR,
                        )
                    # scale+bias from PSUM: alternate Act/DVE so neither
                    # engine bottlenecks phase B
                    if m % 2 == 0:
                        nc.scalar.activation(
                            pre_ctx[:, m, :],
                            ps[:],
                            AF.Identity,
                            scale=INV_PROD,
                            bias=bdec[:, m : m + 1],
                        )
                    else:
                        nc.vector.tensor_scalar(
                            pre_ctx[:, m, :],
                            ps[:],
                            INV_PROD,
                            bdec[:, m : m + 1],
                            op0=ALU.mult,
                            op1=ALU.add,
                        )

        # ---------------- phase C: decoder scan + logits -----------------
        with (
            tc.tile_pool(name=f"{px}cw", bufs=1) as pcw,
            tc.tile_pool(name=f"{px}ctok", bufs=2) as ptok,
            tc.tile_pool(name=f"{px}cws", bufs=12) as pws,
            tc.tile_pool(name=f"{px}cls", bufs=3) as pls,
            tc.tile_pool(name=f"{px}cps", bufs=8, space="PSUM") as ppsc,
        ):
            wtok_sb = pcw.tile([128, 4, 4096], F8, name=f"{px}wtok_sb")
            for k in range(4):
                nc.sync.dma_start(wtok_sb[:, k, :], p_wtok[k])
            wout_sb = pcw.tile([128, 8, 512], BF16, name=f"{px}wout_sb")
            for k in range(8):
                nc.sync.dma_start(wout_sb[:, k, :], p_wout[k])

            for n in range(n_steps):
                h_prev = h_bufs[n % 2]
                h_new = h_bufs[(n + 1) % 2]
                if n > 0:
                    tok = ptok.tile(
                        [128, 4, rows], F8, tag="tok", name=f"{px}tok{n}"
                    )
                    nc.sync.dma_start(tok[:], p_xt[n - 1])
                avs = {}
                tccs = {}

                def emit_chains(hc, n=n, tok=(tok if n > 0 else None)):
                    """4 gate-chain matmul groups + gate assembly for hc."""
                    acts = []
                    for g in range(4):
                        m = g * 8 + hc
                        ps = ppsc.tile(
                            [128, rows], F32, tag="ps", name=f"{px}ps{n}_{hc}_{g}"
                        )
                        # DoubleRow fp8: each matmul contracts a pair of
                        # 128-k-chunks (stationary [128,2,128], moving
                        # [128,2,rows]); PSUM holds 2^17 x the true value.
                        for kp in range(4):
                            nc.tensor.matmul(
                                ps[:],
                                whh_sb[:, 2 * kp : 2 * kp + 2, ts(m, 128)],
                                h_prev[:, 2 * kp : 2 * kp + 2, :],
                                start=(kp == 0),
                                stop=(kp == 3 and n == 0),
                                perf_mode=DR,
                            )
                        if n > 0:
                            for kp in range(2):
                                nc.tensor.matmul(
                                    ps[:],
                                    wtok_sb[:, 2 * kp : 2 * kp + 2, ts(m, 128)],
                                    tok[:, 2 * kp : 2 * kp + 2, :],
                                    start=False,
                                    stop=(kp == 1),
                                    perf_mode=DR,
                                )
                        gs = pws.tile(
                            [128, rows], F32, tag="ws", name=f"{px}gs{n}_{hc}_{g}"
                        )
                        # gate assembly reads PSUM, so it must be on DVE
                        # (GPSIMD/Pool cannot access PSUM); keeping DVE free of
                        # cell math avoids FIFO head-of-line blocking on the
                        # PSUM-freeing ops
                        nc.vector.scalar_tensor_tensor(
                            gs[:],
                            ps[:],
                            INV_PROD,
                            pre_ctx[:, m, :],
                            op0=ALU.mult,
                            op1=ALU.add,
                        )
                        acts.append(gs)
                    return acts

                def emit_avs(hc, gss, n=n):
                    out = []
                    for g, gs in enumerate(gss):
                        m = g * 8 + hc
                        av = pws.tile(
                            [128, rows], F32, tag="ws", name=f"{px}av{n}_{hc}_{g}"
                        )
                        # step 0's token is all-zero: subtract the folded
                        # 0.5*rowsum(W_tok) back out via the activation bias
                        nc.scalar.activation(
                            av[:],
                            gs[:],
                            AF.Tanh if g == 2 else AF.Sigmoid,
                            bias=tokneg[:, m : m + 1] if n == 0 else 0.0,
                        )
                        out.append(av)
                    return out

                def emit_cell_a(hc, n=n):
                    """c update. HW costs: Act 0.45us < DVE 0.63 << Pool 1.17
                    per [128,512] op, and the PE (not elementwise) is the
                    per-step floor — so keep the serial c path on DVE/Act and
                    give Pool only the parallel ig branch."""
                    si, sf, tg, so = avs[hc]
                    fc = pws.tile([128, rows], F32, tag="ws", name=f"{px}fc{n}_{hc}")
                    nc.vector.scalar_tensor_tensor(
                        fc[:],
                        sf[:],
                        0.0,
                        c_dec[:, hc, :],
                        op0=ALU.bypass,
                        op1=ALU.mult,
                    )
                    ig = pws.tile([128, rows], F32, tag="ws", name=f"{px}ig{n}_{hc}")
                    nc.gpsimd.tensor_mul(ig[:], si[:], tg[:])
                    nc.vector.tensor_add(c_dec[:, hc, :], fc[:], ig[:])
                    tcc = pws.tile([128, rows], F32, tag="ws", name=f"{px}tc{n}_{hc}")
                    nc.scalar.activation(tcc[:], c_dec[:, hc, :], AF.Tanh)
                    tccs[hc] = tcc

                def emit_cell_b(hc, n=n):
                    """h writes: bf16 h on DVE (so*tcc), e4m3*64 copy on Act
                    (|h|<1 so 64*h stays far below the e4m3 max 240)."""
                    so = avs[hc][3]
                    tcc = tccs[hc]
                    nc.vector.scalar_tensor_tensor(
                        h_bf[:, hc, :],
                        so[:],
                        0.0,
                        tcc[:],
                        op0=ALU.bypass,
                        op1=ALU.mult,
                    )
                    nc.scalar.activation(
                        h_new[:, hc, :], h_bf[:, hc, :], AF.Identity, scale=H_SCALE
                    )

                # software pipeline over hc groups: chains(hc) | avs(hc) |
                # cell_a(hc-1) | cell_b(hc-2), so no engine's FIFO head ever
                # waits on a dep produced less than a full group earlier
                for hc in range(8):
                    gss = emit_chains(hc)
                    avs[hc] = emit_avs(hc, gss)
                    if hc >= 1:
                        emit_cell_a(hc - 1)
                    if hc >= 2:
                        emit_cell_b(hc - 2)
                emit_cell_a(7)
                emit_cell_b(6)
                emit_cell_b(7)
                # logits_T = out_w @ h_T + out_b, streamed to HBM. k-major
                # order with k=7 last: all four PSUM chains run their 28
                # early matmuls while the last h chunk's cell tail drains,
                # instead of each chain stalling on h_bf[7] in turn.
                psls = [
                    ppsc.tile([128, rows], F32, tag="ps", name=f"{px}lp{n}_{mc}")
                    for mc in range(4)
                ]
                for k in range(8):
                    for mc in range(4):
                        nc.tensor.matmul(
                            psls[mc][:],
                            wout_sb[:, k, ts(mc, 128)],
                            h_bf[:, k, :],
                            start=(k == 0),
                            stop=(k == 7),
                        )
                for mc in range(4):
                    psl = psls[mc]
                    lt = pls.tile([128, rows], BF16, tag="ls", name=f"{px}lt{n}_{mc}")
                    nc.scalar.activation(
                        lt[:], psl[:], AF.Identity, bias=bout[:, mc : mc + 1]
                    )
                    nc.sync.dma_start(p_out[n, ts(mc, 128)], lt[:])


def build_nc(s_steps: int = S, n_steps: int = N, bl: int = BL, repeat: int = 1, a_reps: int = 1):
    rows = s_steps * bl  # decoder row count (s, b) per core
    # Bacc (not plain Bass): its compile pipeline splits multi-sem waits into
    # event-semaphore instructions, which walrus codegen requires on TRN2.
    nc = bacc.Bacc("TRN2", target_bir_lowering=False)

    prm = (
        nc.declare_dram_parameter("zt", [128, 4, bl], BF16, isOutput=False),
        nc.declare_dram_parameter(
            "xt", [n_steps - 1, 128, 4, rows], F8, isOutput=False
        ),
        nc.declare_dram_parameter("wlin", [4, 128, 2048], BF16, isOutput=False),
        nc.declare_dram_parameter("blin", [128, 16], F32, isOutput=False),
        nc.declare_dram_parameter("wcond", [8, 128, 4096], F8, isOutput=False),
        nc.declare_dram_parameter("bcond", [1, 4096], BF16, isOutput=False),
        nc.declare_dram_parameter("wpre", [8, 128, 2048], F8, isOutput=False),
        nc.declare_dram_parameter("bpre", [128, 16], F32, isOutput=False),
        nc.declare_dram_parameter("wctx", [32, 128, 2048], F8, isOutput=False),
        nc.declare_dram_parameter("bdec", [128, 32], F32, isOutput=False),
        nc.declare_dram_parameter("wtok", [4, 128, 4096], F8, isOutput=False),
        nc.declare_dram_parameter("whh", [8, 128, 4096], F8, isOutput=False),
        nc.declare_dram_parameter("wout", [8, 128, 512], BF16, isOutput=False),
        nc.declare_dram_parameter("bout", [128, 4], F32, isOutput=False),
        nc.declare_dram_parameter("tokneg", [128, 32], F32, isOutput=False),
        nc.declare_dram_parameter("bpre64", [128, 16], F32, isOutput=False),
        nc.declare_dram_parameter("out", [n_steps, 512, rows], BF16, isOutput=True),
    )

    with tile.TileContext(nc) as tc:
        for rep in range(repeat):
            _emit(nc, tc, prm, s_steps, n_steps, bl, f"r{rep}_" if repeat > 1 else "", a_reps=a_reps)
    # Run the Bacc compile pipeline (register allocation, wait splitting) —
    # run_bass_via_pjrt serializes the module as-is and walrus needs this.
    nc.finalize()
    return nc


def _chunk_t(w: np.ndarray, kc: int) -> np.ndarray:
    """[M, K] weight -> transposed bf16 chunks [kc, 128, M]."""
    m, k = w.shape
    assert k == kc * 128
    return np.ascontiguousarray(w.T.reshape(kc, 128, m)).astype(ml_dtypes.bfloat16)


def _chunk_q8(w: np.ndarray, kc: int, scale: float) -> np.ndarray:
    """[M, K] weight -> transposed e4m3 chunks [kc, 128, M] at `scale`."""
    m, k = w.shape
    assert k == kc * 128
    wt = np.ascontiguousarray(w.T.reshape(kc, 128, m)).astype(np.float32) * scale
    return np.clip(wt, -240.0, 240.0).astype(ml_dtypes.float8_e4m3)


def _bias_cols(b: np.ndarray, nch: int) -> np.ndarray:
    """[nch*128] bias -> [128, nch] fp32 (column m = chunk m)."""
    return np.ascontiguousarray(b.reshape(nch, 128).T).astype(np.float32)


def pack_shared(
    lin_in_w, lin_in_b, cond_Whh, cond_bih, cond_bhh,
    pre_w, pre_b, dec_Wih, dec_Whh, dec_bih, dec_bhh, out_w, out_b,
) -> dict:
    wctx = dec_Wih[:, : 2 * LH]  # [4096, 2048]
    # [m-chunk, partition(k within chunk), (k-chunk, m-col)] so each m-chunk
    # loads with a single contiguous DMA; e4m3 at W_SCALE for DoubleRow
    wctx_p = np.clip(
        wctx.T.reshape(16, 128, 32, 128).transpose(2, 1, 0, 3).reshape(32, 128, 2048)
        .astype(np.float32) * W_SCALE,
        -240.0,
        240.0,
    ).astype(ml_dtypes.float8_e4m3)
    w_tok = dec_Wih[:, 2 * LH :].astype(np.float32)  # [4096, T]
    # tokens are centered on-chip: fold 0.5*rowsum(W_tok) into the pre_ctx
    # bias; step 0 (zero token) subtracts it back via the gate activation bias
    tok_bias = 0.5 * w_tok.sum(axis=1)
    return {
        "wlin": _chunk_t(lin_in_w, 4),
        "blin": _bias_cols(lin_in_b, 16),
        "wcond": _chunk_q8(cond_Whh, 8, W_SCALE),
        "bcond": ((cond_bih + cond_bhh) * np.float32(W_SCALE))
        .astype(ml_dtypes.bfloat16).reshape(1, 4 * CH),
        "wpre": _chunk_q8(pre_w, 8, W_SCALE),
        "bpre": _bias_cols(pre_b, 16),
        "wctx": wctx_p,
        "bdec": _bias_cols(dec_bih + dec_bhh + tok_bias, 32),
        "wtok": _chunk_q8(w_tok, 4, W_SCALE),
        "whh": _chunk_q8(dec_Whh, 8, W_SCALE),
        "wout": _chunk_t(out_w, 8),
        "bout": _bias_cols(out_b, 4),
        "tokneg": _bias_cols(-tok_bias, 32),
        "bpre64": _bias_cols(pre_b * np.float32(H_SCALE), 16),
    }


def pack_data(z: np.ndarray, x: np.ndarray, s_steps=S, n_steps=N):
    """Returns per-core zt [128,4,bl] bf16 and xt [n-1,128,4,rows] e4m3.

    Partition dim second so each tensor loads as one contiguous DMA:
    zt[p, kc, b] = z[b, kc*128+p]; xt[j, p, kc, (s, b)] = tok slab j at
    t = kc*128+p (slab j feeds decoder step j+1). Tokens are centered
    (tok - 0.5) and scaled by 64 for the fp8 DoubleRow matmuls.
    """
    b = z.shape[0]
    bl = b // NCORES
    zt = z.T.reshape(4, 128, b).transpose(1, 0, 2).astype(ml_dtypes.bfloat16)
    # x [B, S*N, T] -> [N, T, S, B] -> [N, p, kc, S, B]
    xr = np.ascontiguousarray(
        (x.reshape(b, s_steps, n_steps, T).transpose(2, 3, 1, 0)[: n_steps - 1]
         .astype(np.float32) - 0.5) * np.float32(H_SCALE)
    ).astype(ml_dtypes.float8_e4m3)
    xt = xr.reshape(n_steps - 1, 4, 128, s_steps, b).transpose(0, 2, 1, 3, 4)
    zts, xts = [], []
    for c in range(NCORES):
        sl = slice(c * bl, (c + 1) * bl)
        zts.append(np.ascontiguousarray(zt[:, :, sl]))
        xts.append(
            np.ascontiguousarray(xt[..., sl]).reshape(
                n_steps - 1, 128, 4, s_steps * bl
            )
        )
    return zts, xts


_NC_CACHE = {}


def kernel(z, x, lin_in_w, lin_in_b, cond_Wih, cond_Whh, cond_bih, cond_bhh,
           pre_w, pre_b, dec_Wih, dec_Whh, dec_bih, dec_bhh, out_w, out_b):
    from concourse.bass_utils import run_bass_kernel_spmd

    args = [z, x, lin_in_w, lin_in_b, cond_Wih, cond_Whh, cond_bih, cond_bhh,
            pre_w, pre_b, dec_Wih, dec_Whh, dec_bih, dec_bhh, out_w, out_b]
    (z, x, lin_in_w, lin_in_b, cond_Wih, cond_Whh, cond_bih, cond_bhh,
     pre_w, pre_b, dec_Wih, dec_Whh, dec_bih, dec_bhh, out_w, out_b) = [
        np.asarray(a, dtype=np.float32) for a in args
    ]

    if "nc" not in _NC_CACHE:
        _NC_CACHE["nc"] = build_nc()
    nc = _NC_CACHE["nc"]

    shared = pack_shared(
        lin_in_w, lin_in_b, cond_Whh, cond_bih, cond_bhh,
        pre_w, pre_b, dec_Wih, dec_Whh, dec_bih, dec_bhh, out_w, out_b,
    )
    zts, xts = pack_data(z, x)
    in_maps = [{**shared, "zt": zts[c], "xt": xts[c]} for c in range(NCORES)]

    res = run_bass_kernel_spmd(nc, in_maps, list(range(NCORES)))

    out = np.empty((B, S, N, T), dtype=np.float32)
    for c in range(NCORES):
        # per-core out [N, T, S*bl] -> [bl, S, N, T]
        oc = (res.results[c]["out"].astype(np.float32)
              .reshape(N, T, S, BL).transpose(3, 2, 0, 1))
        out[c * BL : (c + 1) * BL] = oc
    return out.reshape(B, S * N, T)



# revision 57
# speedup vs baseline: 1.0558x; 1.0558x over previous
"""Trainium2 Bass kernel for nn_Decoder (conductor-LSTM -> decoder-LSTM -> logits).

Sharding: pure data-parallel over batch B=256 -> 32 per core on 8 NeuronCores.
No collectives; each core runs an identical program on its batch slice.

On-chip layouts are "transposed": feature dims live on SBUF partitions, the
(s, b) row index lives on the free dim, so decoder matmuls are weight-stationary
[128,128] x [128,512] with fp32 PSUM accumulation.

Precision: the decoder scan (85% of FLOPs) and the pre-decoder (pre + ctx)
matmuls run in fp8e4 DoubleRow mode (2 contraction elements per PE cell,
~1.4-1.8x bf16 throughput): weights pre-quantized e4m3 at scale 2048,
h / dec_in / centered tokens at scale 64; the 2^-17 product scale is folded
into the gate-assembly op or the psum-reading activation. Teacher-forcing
tokens are centered (tok - 0.5) with 0.5*rowsum(W_tok) folded into pre_ctx,
which halves fp8 quantization noise; step 0 (zero token) undoes that fold
exactly via the gate activation bias. The logits matmul and the conductor
stay bf16 (their error hits the output directly / compounds); LSTM cell
state c and all PSUM accumulation stay fp32. Logits are written to HBM as
bf16 (host converts back to fp32) - the fp32 output drain cost ~72 us.
Measured rel err 1.659e-2 (emulator-exact; gate is 2e-2), HW 1.198 ms vs
the 2.100 ms bf16 baseline on the same slope methodology.

Scheduling: the decoder loop is software-pipelined over hidden-chunk groups
(chains+gate-assembly | activations | cell | h-writes lagged by 0/0/1/2
groups) so no strict-FIFO engine ever head-of-line blocks the PSUM-draining
ops that pace the PE; logits matmuls are k-major with the freshest h chunk
last. Elementwise ops are placed by measured HW cost (Act 0.45us < DVE
0.63us << Pool/GPSIMD 1.17us per [128,512] op; GPSIMD only runs plain
TensorTensor and cannot touch PSUM). The conductor folds its bias in as a
rank-1 ones x bias matmul, groups transposes per gate to cut 32 tiny
activations to 4, and runs a tiny dependent matmul mid-cell to keep the PE
HAM clock-gate from re-throttling during the serial cell gap.
"""

import os
import sys

for _p in ("/opt/trn_rl_repo", "/root/.axon_site/_ro/trn_rl_repo"):
    if os.path.isdir(_p) and _p not in sys.path:
        sys.path.insert(0, _p)

import ml_dtypes
import numpy as np

import concourse.bass as bass
import concourse.mybir as mybir
import concourse.tile as tile
from concourse import bacc
from concourse.bass import ts
from concourse.masks import make_identity

F32 = mybir.dt.float32
BF16 = mybir.dt.bfloat16
F8 = mybir.dt.float8e4
DR = mybir.MatmulPerfMode.DoubleRow
AF = mybir.ActivationFunctionType
ALU = mybir.AluOpType

W_SCALE = 2048.0  # e4m3 scale for decoder weights
H_SCALE = 64.0  # e4m3 scale for h and centered tokens
INV_PROD = 1.0 / (W_SCALE * H_SCALE)  # 2^-17, folded into gate assembly

B, T, Z, CH, LH, S, N = 256, 512, 512, 1024, 1024, 16, 16
NCORES = 8
BL = B // NCORES  # 32 batch rows per core


def _emit(nc, tc, prm, s_steps, n_steps, bl, px, a_reps=1):
    """Trace one full forward pass. px prefixes pool/tile names."""
    rows = s_steps * bl
    (p_zt, p_xt, p_wlin, p_blin, p_wcond, p_bcond, p_wpre, p_bpre, p_wctx,
     p_bdec, p_wtok, p_whh, p_wout, p_bout, p_tokneg, p_bpre64, p_out) = prm

    with (
        tc.tile_pool(name=f"{px}const", bufs=1) as pconst,
        tc.tile_pool(name=f"{px}state", bufs=1) as pstate,
    ):
        ident = pconst.tile([32, 32], F32, name=f"{px}ident")
        make_identity(nc, ident[:])
        blin = pconst.tile([128, 16], F32, name=f"{px}blin")
        nc.sync.dma_start(blin[:], p_blin[:])
        # conductor bias as a bf16 row: added into the gate PSUM via a
        # rank-1 (ones x bias) matmul appended to each wave chain
        bcond = pconst.tile([1, 4096], BF16, name=f"{px}bcond")
        nc.sync.dma_start(bcond[:], p_bcond[:])
        ones32 = pconst.tile([1, 32], BF16, name=f"{px}ones32")
        nc.vector.memset(ones32[:], 1.0)
        bpre = pconst.tile([128, 16], F32, name=f"{px}bpre")
        nc.sync.dma_start(bpre[:], p_bpre[:])
        bdec = pconst.tile([128, 32], F32, name=f"{px}bdec")
        nc.sync.dma_start(bdec[:], p_bdec[:])
        bout = pconst.tile([128, 4], F32, name=f"{px}bout")
        nc.sync.dma_start(bout[:], p_bout[:])
        tokneg = pconst.tile([128, 32], F32, name=f"{px}tokneg")
        nc.sync.dma_start(tokneg[:], p_tokneg[:])
        bpre64 = pconst.tile([128, 16], F32, name=f"{px}bpre64")
        nc.sync.dma_start(bpre64[:], p_bpre64[:])

        # Persistent decoder state. The fp8 h (scale 64, feeds the DoubleRow
        # gate matmuls) is double-buffered: within a step every hidden-chunk
        # matmul reads all 8 chunks of the old h, so the cell must not
        # overwrite them in place. h_bf is the bf16 h for the logits matmul.
        c_dec = pstate.tile([128, 8, rows], F32, name=f"{px}c_dec")
        h_bufs = [
            pstate.tile([128, 8, rows], F8, name=f"{px}h_dec{i}") for i in range(2)
        ]
        h_bf = pstate.tile([128, 8, rows], BF16, name=f"{px}h_bf")
        pre_ctx = pstate.tile([128, 32, rows], BF16, name=f"{px}pre_ctx")

        with tc.tile_pool(name=f"{px}ab", bufs=1) as pab:
            # conductor h history, laid out [p, k-chunk, s, b] so that
            # [:, k, s, :] is a [128, bl] matmul operand and [:, k] is a
            # contiguous [128, s*bl] rhs for the pre-decoder matmuls.
            # The bf16 copy drives the conductor recurrence; the e4m3*64
            # copy feeds phase B's fp8 DoubleRow pre-matmuls.
            h_all = pab.tile([128, 8, s_steps, bl], BF16, name=f"{px}h_all")
            h_all8 = pab.tile([128, 8, s_steps, bl], F8, name=f"{px}h_all8")

            # ---------------- phase A: linear_in + conductor scan ------
            with (
                tc.tile_pool(name=f"{px}aw", bufs=1) as paw,
                tc.tile_pool(name=f"{px}ag", bufs=8) as pgsb,
                tc.tile_pool(name=f"{px}acell", bufs=2) as pcell,
                tc.tile_pool(name=f"{px}aps", bufs=4, space="PSUM") as ppsa,
                tc.tile_pool(name=f"{px}atp", bufs=4, space="PSUM") as ptpa,
            ):
                wlin_sb = paw.tile([128, 4, 2048], BF16, name=f"{px}wlin_sb")
                for k in range(4):
                    nc.sync.dma_start(wlin_sb[:, k, :], p_wlin[k])
                # zt before the big wcond load so the lin_in chains can start
                # as soon as wlin+zt land (wcond is only needed ~8 matmuls in)
                zt_sb = paw.tile([128, 4, bl], BF16, name=f"{px}zt_sb")
                nc.sync.dma_start(zt_sb[:], p_zt[:])
                wcond_sb = paw.tile([128, 8, 4096], BF16, name=f"{px}wcond_sb")
                for k in range(8):
                    nc.sync.dma_start(wcond_sb[:, k, :], p_wcond[k])

                h0 = paw.tile([128, 8, bl], BF16, name=f"{px}h0")
                c_cond = paw.tile([128, 8, bl], F32, name=f"{px}c_cond")

                # hc0_T = tanh(lin_in_w @ z_T + b): chunks 0-7 -> h0,
                # chunks 8-15 -> c0
                for m in range(16):
                    ps = ppsa.tile([128, bl], F32, tag="mm", name=f"{px}aps{m}")
                    for k in range(4):
                        nc.tensor.matmul(
                            ps[:],
                            wlin_sb[:, k, ts(m, 128)],
                            zt_sb[:, k, :],
                            start=(k == 0),
                            stop=(k == 3),
                        )
                    dst = h0[:, m, :] if m < 8 else c_cond[:, m - 8, :]
                    nc.scalar.activation(dst, ps[:], AF.Tanh, bias=blin[:, m : m + 1])

                # conductor: gates = h @ Whh.T + bias (input term is zero).
                # a_reps > 1 repeats the whole conductor pass (timing-only
                # builds, for differential phase-A measurement).
                for s in range(s_steps * a_reps):

                    def hsrc(k, s=s):
                        return (
                            h0[:, k, :]
                            if s == 0
                            else h_all[:, k, (s - 1) % s_steps, :]
                        )

                    si = pcell.tile([128, 8, bl], F32, tag="si", name=f"{px}si{s}")
                    sf = pcell.tile([128, 8, bl], F32, tag="sf", name=f"{px}sf{s}")
                    tg = pcell.tile([128, 8, bl], F32, tag="tg", name=f"{px}tg{s}")
                    so = pcell.tile([128, 8, bl], F32, tag="so", name=f"{px}so{s}")
                    # 2 waves of 4 column-group-tiled matmul chains: the four
                    # M=32 chains run concurrently in distinct 32-col strips
                    # of the PE array, packing the gates for 4 n-chunks into
                    # one [128, 512] psum tile. The conductor bias rides in as
                    # a rank-1 (ones x bias_row) matmul at the end of each
                    # chain, so the activations need no per-chunk bias.
                    for wave in range(2):
                        ps = ppsa.tile(
                            [128, 512], F32, tag="mm", name=f"{px}cps{s}_{wave}"
                        )
                        for k in range(8):
                            for j in range(4):
                                # skip_group_check: the sim's zero-region
                                # conflict guard is partition-base-blind; the
                                # four chains write disjoint 32-row slices.
                                nc.tensor.matmul(
                                    ps[ts(j, bl), :],
                                    hsrc(k),
                                    wcond_sb[:, k, ts(wave * 4 + j, 512)],
                                    start=(k == 0),
                                    stop=False,
                                    tile_position=(0, j * 32),
                                    skip_group_check=True,
                                )
                        for j in range(4):
                            nch = wave * 4 + j
                            nc.tensor.matmul(
                                ps[ts(j, bl), :],
                                ones32[:, :],
                                bcond[:, ts(nch, 512)],
                                start=False,
                                stop=True,
                                tile_position=(0, j * 32),
                                skip_group_check=True,
                            )
                        gtiles = []
                        for j in range(4):
                            nch = wave * 4 + j
                            g = pgsb.tile(
                                [bl, 512], F32, tag="g", name=f"{px}g{s}_{nch}"
                            )
                            nc.vector.tensor_copy(g[:], ps[ts(j, bl), :])
                            gtiles.append(g)
                        # transpose into per-gate [128, 8*32] staging, then a
                        # single activation per gate (gate g is complete
                        # within one wave: its 8 hc-chunks span two j strips)
                        for gl in range(2):
                            gate = wave * 2 + gl
                            tpg = ptpa.tile(
                                [128, 8, bl], F32, tag="tp", name=f"{px}tp{s}_{gate}"
                            )
                            for idx in range(8):
                                j = gl * 2 + idx // 4
                                jj = idx % 4
                                nc.tensor.transpose(
                                    tpg[:, idx, :],
                                    gtiles[j][:, ts(jj, 128)],
                                    ident[:],
                                )
                            dstt = (si, sf, tg, so)[gate]
                            fn = AF.Tanh if gate == 2 else AF.Sigmoid
                            nc.scalar.activation(dstt[:, :, :], tpg[:], fn)
                    fc = pcell.tile([128, 8, bl], F32, tag="fc", name=f"{px}fc{s}")
                    nc.vector.scalar_tensor_tensor(
                        fc[:], sf[:], 0.0, c_cond[:], op0=ALU.bypass, op1=ALU.mult
                    )
                    ig = pcell.tile([128, 8, bl], F32, tag="ig", name=f"{px}ig{s}")
                    nc.vector.scalar_tensor_tensor(
                        ig[:], si[:], 0.0, tg[:], op0=ALU.bypass, op1=ALU.mult
                    )
                    nc.vector.tensor_add(c_cond[:], fc[:], ig[:])
                    tcc = pcell.tile([128, 8, bl], F32, tag="tcc", name=f"{px}tcc{s}")
                    nc.scalar.activation(tcc[:], c_cond[:], AF.Tanh)

                    nc.vector.scalar_tensor_tensor(
                        h_all[:, :, s % s_steps, :],
                        so[:],
                        0.0,
                        tcc[:],
                        op0=ALU.bypass,
                        op1=ALU.mult,
                    )
                    nc.vector.scalar_tensor_tensor(
                        h_all8[:, :, s % s_steps, :],
                        so[:],
                        H_SCALE,
                        tcc[:],
                        op0=ALU.mult,
                        op1=ALU.mult,
                    )
                    # HAM warm-keeper: a tiny matmul that depends on mid-gap
                    # data (the fresh h slice) splits the PE-idle window below
                    # the ~3.4us re-throttle threshold, keeping the conductor
                    # matmuls at 2.4 GHz
                    warm = ppsa.tile([32, 32], F32, tag="mm", name=f"{px}warm{s}")
                    nc.tensor.matmul(
                        warm[:],
                        ones32[:, :],
                        h_all[0:1, 0, s % s_steps, :],
                        start=True,
                        stop=True,
                    )

            # ---------------- phase B: pre-decoder -----------------------
            with (
                tc.tile_pool(name=f"{px}bw", bufs=1) as pbw,
                tc.tile_pool(name=f"{px}bctx", bufs=4) as pbctx,
                tc.tile_pool(name=f"{px}bps", bufs=8, space="PSUM") as ppsb,
            ):
                wpre_sb = pbw.tile([128, 8, 2048], F8, name=f"{px}wpre_sb")
                for k in range(8):
                    nc.sync.dma_start(wpre_sb[:, k, :], p_wpre[k])
                # dec_in quantized e4m3 at scale 64 (max|dec_in| ~0.44, so
                # 64x stays far below 240) for the fp8 DoubleRow ctx matmuls
                dec_in = pbw.tile([128, 16, rows], F8, name=f"{px}dec_in")

                # dec_in_T = pre_w @ cond_outs_T + pre_b (fp8 DoubleRow; the
                # PSUM carries 2^17 x the true value)
                for m in range(16):
                    ps = ppsb.tile([128, rows], F32, tag="ps", name=f"{px}bps{m}")
                    for kp in range(4):
                        nc.tensor.matmul(
                            ps[:],
                            wpre_sb[:, 2 * kp : 2 * kp + 2, ts(m, 128)],
                            h_all8[:, 2 * kp : 2 * kp + 2],
                            start=(kp == 0),
                            stop=(kp == 3),
                            perf_mode=DR,
                        )
                    nc.scalar.activation(
                        dec_in[:, m, :],
                        ps[:],
                        AF.Identity,
                        scale=H_SCALE * INV_PROD,
                        bias=bpre64[:, m : m + 1],
                    )
                    if m < 8:
                        # decoder h0, e4m3 at scale 64 (same value as dec_in)
                        nc.scalar.activation(
                            h_bufs[0][:, m, :],
                            ps[:],
                            AF.Identity,
                            scale=H_SCALE * INV_PROD,
                            bias=bpre64[:, m : m + 1],
                        )
                    else:
                        # decoder c0 (fp32): ps*2^-17 + pre_b
                        nc.vector.tensor_scalar(
                            c_dec[:, m - 8, :],
                            ps[:],
                            INV_PROD,
                            bpre[:, m : m + 1],
                            op0=ALU.mult,
                            op1=ALU.add,
                        )

                # pre_ctx_T = W_ctx @ dec_in_T + dec_bias (constant over n),
                # fp8 DoubleRow pairs over the 16 k-chunks
                for m in range(32):
                    wt = pbctx.tile(
                        [128, 16, 128], F8, tag="wctx", name=f"{px}wc{m}"
                    )
                    nc.sync.dma_start(wt[:], p_wctx[m])
                    ps = ppsb.tile([128, rows], F32, tag="ps", name=f"{px}xps{m}")
                    for kp in range(8):
                        nc.tensor.matmul(
                            ps[:],
                            wt[:, 2 * kp : 2 * kp + 2, :],
                            dec_in[:, 2 * kp : 2 * kp + 2, :],
                            start=(kp == 0),
                            stop=(kp == 7),
                            perf_mode=DR,
                        )
                    # scale+bias from PSUM: alternate Act/DVE so neither
                    # engine bottlenecks phase B
                    if m % 2 == 0:
                        nc.scalar.activation(
                            pre_ctx[:, m, :],
                            ps[:],
                            AF.Identity,
                            scale=INV_PROD,
                            bias=bdec[:, m : m + 1],
                        )
                    else:
                        nc.vector.tensor_scalar(
                            pre_ctx[:, m, :],
                            ps[:],
                            INV_PROD,
                            bdec[:, m : m + 1],
                            op0=ALU.mult,
                            op1=ALU.add,
                        )

        # ---------------- phase C: decoder scan + logits -----------------
        with (
            tc.tile_pool(name=f"{px}cw", bufs=1) as pcw,
            tc.tile_pool(name=f"{px}ctok", bufs=2) as ptok,
            tc.tile_pool(name=f"{px}cws", bufs=12) as pws,
            tc.tile_pool(name=f"{px}cls", bufs=3) as pls,
            tc.tile_pool(name=f"{px}cps", bufs=8, space="PSUM") as ppsc,
        ):
            whh_sb = pcw.tile([128, 8, 4096], F8, name=f"{px}whh_sb")
            for k in range(8):
                nc.sync.dma_start(whh_sb[:, k, :], p_whh[k])
            wtok_sb = pcw.tile([128, 4, 4096], F8, name=f"{px}wtok_sb")
            for k in range(4):
                nc.sync.dma_start(wtok_sb[:, k, :], p_wtok[k])
            wout_sb = pcw.tile([128, 8, 512], BF16, name=f"{px}wout_sb")
            for k in range(8):
                nc.sync.dma_start(wout_sb[:, k, :], p_wout[k])

            for n in range(n_steps):
                h_prev = h_bufs[n % 2]
                h_new = h_bufs[(n + 1) % 2]
                if n > 0:
                    tok = ptok.tile(
                        [128, 4, rows], F8, tag="tok", name=f"{px}tok{n}"
                    )
                    nc.sync.dma_start(tok[:], p_xt[n - 1])
                avs = {}
                tccs = {}

                def emit_chains(hc, n=n, tok=(tok if n > 0 else None)):
                    """4 gate-chain matmul groups + gate assembly for hc."""
                    acts = []
                    for g in range(4):
                        m = g * 8 + hc
                        ps = ppsc.tile(
                            [128, rows], F32, tag="ps", name=f"{px}ps{n}_{hc}_{g}"
                        )
                        # DoubleRow fp8: each matmul contracts a pair of
                        # 128-k-chunks (stationary [128,2,128], moving
                        # [128,2,rows]); PSUM holds 2^17 x the true value.
                        for kp in range(4):
                            nc.tensor.matmul(
                                ps[:],
                                whh_sb[:, 2 * kp : 2 * kp + 2, ts(m, 128)],
                                h_prev[:, 2 * kp : 2 * kp + 2, :],
                                start=(kp == 0),
                                stop=(kp == 3 and n == 0),
                                perf_mode=DR,
                            )
                        if n > 0:
                            for kp in range(2):
                                nc.tensor.matmul(
                                    ps[:],
                                    wtok_sb[:, 2 * kp : 2 * kp + 2, ts(m, 128)],
                                    tok[:, 2 * kp : 2 * kp + 2, :],
                                    start=False,
                                    stop=(kp == 1),
                                    perf_mode=DR,
                                )
                        gs = pws.tile(
                            [128, rows], F32, tag="ws", name=f"{px}gs{n}_{hc}_{g}"
                        )
                        # gate assembly reads PSUM, so it must be on DVE
                        # (GPSIMD/Pool cannot access PSUM); keeping DVE free of
                        # cell math avoids FIFO head-of-line blocking on the
                        # PSUM-freeing ops
                        nc.vector.scalar_tensor_tensor(
                            gs[:],
                            ps[:],
                            INV_PROD,
                            pre_ctx[:, m, :],
                            op0=ALU.mult,
                            op1=ALU.add,
                        )
                        acts.append(gs)
                    return acts

                def emit_avs(hc, gss, n=n):
                    out = []
                    for g, gs in enumerate(gss):
                        m = g * 8 + hc
                        av = pws.tile(
                            [128, rows], F32, tag="ws", name=f"{px}av{n}_{hc}_{g}"
                        )
                        # step 0's token is all-zero: subtract the folded
                        # 0.5*rowsum(W_tok) back out via the activation bias
                        nc.scalar.activation(
                            av[:],
                            gs[:],
                            AF.Tanh if g == 2 else AF.Sigmoid,
                            bias=tokneg[:, m : m + 1] if n == 0 else 0.0,
                        )
                        out.append(av)
                    return out

                def emit_cell_a(hc, n=n):
                    """c update. HW costs: Act 0.45us < DVE 0.63 << Pool 1.17
                    per [128,512] op, and the PE (not elementwise) is the
                    per-step floor — so keep the serial c path on DVE/Act and
                    give Pool only the parallel ig branch."""
                    si, sf, tg, so = avs[hc]
                    fc = pws.tile([128, rows], F32, tag="ws", name=f"{px}fc{n}_{hc}")
                    nc.vector.scalar_tensor_tensor(
                        fc[:],
                        sf[:],
                        0.0,
                        c_dec[:, hc, :],
                        op0=ALU.bypass,
                        op1=ALU.mult,
                    )
                    ig = pws.tile([128, rows], F32, tag="ws", name=f"{px}ig{n}_{hc}")
                    nc.gpsimd.tensor_mul(ig[:], si[:], tg[:])
                    nc.vector.tensor_add(c_dec[:, hc, :], fc[:], ig[:])
                    tcc = pws.tile([128, rows], F32, tag="ws", name=f"{px}tc{n}_{hc}")
                    nc.scalar.activation(tcc[:], c_dec[:, hc, :], AF.Tanh)
                    tccs[hc] = tcc

                def emit_cell_b(hc, n=n):
                    """h writes: bf16 h on DVE (so*tcc), e4m3*64 copy on Act
                    (|h|<1 so 64*h stays far below the e4m3 max 240)."""
                    so = avs[hc][3]
                    tcc = tccs[hc]
                    nc.vector.scalar_tensor_tensor(
                        h_bf[:, hc, :],
                        so[:],
                        0.0,
                        tcc[:],
                        op0=ALU.bypass,
                        op1=ALU.mult,
                    )
                    nc.scalar.activation(
                        h_new[:, hc, :], h_bf[:, hc, :], AF.Identity, scale=H_SCALE
                    )

                # software pipeline over hc groups: chains(hc) | avs(hc) |
                # cell_a(hc-1) | cell_b(hc-2), so no engine's FIFO head ever
                # waits on a dep produced less than a full group earlier
                for hc in range(8):
                    gss = emit_chains(hc)
                    avs[hc] = emit_avs(hc, gss)
                    if hc >= 1:
                        emit_cell_a(hc - 1)
                    if hc >= 2:
                        emit_cell_b(hc - 2)
                emit_cell_a(7)
                emit_cell_b(6)
                emit_cell_b(7)
                # logits_T = out_w @ h_T + out_b, streamed to HBM. k-major
                # order with k=7 last: all four PSUM chains run their 28
                # early matmuls while the last h chunk's cell tail drains,
                # instead of each chain stalling on h_bf[7] in turn.
                psls = [
                    ppsc.tile([128, rows], F32, tag="ps", name=f"{px}lp{n}_{mc}")
                    for mc in range(4)
                ]
                for k in range(8):
                    for mc in range(4):
                        nc.tensor.matmul(
                            psls[mc][:],
                            wout_sb[:, k, ts(mc, 128)],
                            h_bf[:, k, :],
                            start=(k == 0),
                            stop=(k == 7),
                        )
                for mc in range(4):
                    psl = psls[mc]
                    lt = pls.tile([128, rows], BF16, tag="ls", name=f"{px}lt{n}_{mc}")
                    nc.scalar.activation(
                        lt[:], psl[:], AF.Identity, bias=bout[:, mc : mc + 1]
                    )
                    nc.sync.dma_start(p_out[n, ts(mc, 128)], lt[:])


def build_nc(s_steps: int = S, n_steps: int = N, bl: int = BL, repeat: int = 1, a_reps: int = 1):
    rows = s_steps * bl  # decoder row count (s, b) per core
    # Bacc (not plain Bass): its compile pipeline splits multi-sem waits into
    # event-semaphore instructions, which walrus codegen requires on TRN2.
    nc = bacc.Bacc("TRN2", target_bir_lowering=False)

    prm = (
        nc.declare_dram_parameter("zt", [128, 4, bl], BF16, isOutput=False),
        nc.declare_dram_parameter(
            "xt", [n_steps - 1, 128, 4, rows], F8, isOutput=False
        ),
        nc.declare_dram_parameter("wlin", [4, 128, 2048], BF16, isOutput=False),
        nc.declare_dram_parameter("blin", [128, 16], F32, isOutput=False),
        nc.declare_dram_parameter("wcond", [8, 128, 4096], BF16, isOutput=False),
        nc.declare_dram_parameter("bcond", [1, 4096], BF16, isOutput=False),
        nc.declare_dram_parameter("wpre", [8, 128, 2048], F8, isOutput=False),
        nc.declare_dram_parameter("bpre", [128, 16], F32, isOutput=False),
        nc.declare_dram_parameter("wctx", [32, 128, 2048], F8, isOutput=False),
        nc.declare_dram_parameter("bdec", [128, 32], F32, isOutput=False),
        nc.declare_dram_parameter("wtok", [4, 128, 4096], F8, isOutput=False),
        nc.declare_dram_parameter("whh", [8, 128, 4096], F8, isOutput=False),
        nc.declare_dram_parameter("wout", [8, 128, 512], BF16, isOutput=False),
        nc.declare_dram_parameter("bout", [128, 4], F32, isOutput=False),
        nc.declare_dram_parameter("tokneg", [128, 32], F32, isOutput=False),
        nc.declare_dram_parameter("bpre64", [128, 16], F32, isOutput=False),
        nc.declare_dram_parameter("out", [n_steps, 512, rows], BF16, isOutput=True),
    )

    with tile.TileContext(nc) as tc:
        for rep in range(repeat):
            _emit(nc, tc, prm, s_steps, n_steps, bl, f"r{rep}_" if repeat > 1 else "", a_reps=a_reps)
    # Run the Bacc compile pipeline (register allocation, wait splitting) —
    # run_bass_via_pjrt serializes the module as-is and walrus needs this.
    nc.finalize()
    return nc


def _chunk_t(w: np.ndarray, kc: int) -> np.ndarray:
    """[M, K] weight -> transposed bf16 chunks [kc, 128, M]."""
    m, k = w.shape
    assert k == kc * 128
    return np.ascontiguousarray(w.T.reshape(kc, 128, m)).astype(ml_dtypes.bfloat16)


def _chunk_q8(w: np.ndarray, kc: int, scale: float) -> np.ndarray:
    """[M, K] weight -> transposed e4m3 chunks [kc, 128, M] at `scale`."""
    m, k = w.shape
    assert k == kc * 128
    wt = np.ascontiguousarray(w.T.reshape(kc, 128, m)).astype(np.float32) * scale
    return np.clip(wt, -240.0, 240.0).astype(ml_dtypes.float8_e4m3)


def _bias_cols(b: np.ndarray, nch: int) -> np.ndarray:
    """[nch*128] bias -> [128, nch] fp32 (column m = chunk m)."""
    return np.ascontiguousarray(b.reshape(nch, 128).T).astype(np.float32)


def pack_shared(
    lin_in_w, lin_in_b, cond_Whh, cond_bih, cond_bhh,
    pre_w, pre_b, dec_Wih, dec_Whh, dec_bih, dec_bhh, out_w, out_b,
) -> dict:
    wctx = dec_Wih[:, : 2 * LH]  # [4096, 2048]
    # [m-chunk, partition(k within chunk), (k-chunk, m-col)] so each m-chunk
    # loads with a single contiguous DMA; e4m3 at W_SCALE for DoubleRow
    wctx_p = np.clip(
        wctx.T.reshape(16, 128, 32, 128).transpose(2, 1, 0, 3).reshape(32, 128, 2048)
        .astype(np.float32) * W_SCALE,
        -240.0,
        240.0,
    ).astype(ml_dtypes.float8_e4m3)
    w_tok = dec_Wih[:, 2 * LH :].astype(np.float32)  # [4096, T]
    # tokens are centered on-chip: fold 0.5*rowsum(W_tok) into the pre_ctx
    # bias; step 0 (zero token) subtracts it back via the gate activation bias
    tok_bias = 0.5 * w_tok.sum(axis=1)
    return {
        "wlin": _chunk_t(lin_in_w, 4),
        "blin": _bias_cols(lin_in_b, 16),
        "wcond": _chunk_t(cond_Whh, 8),
        "bcond": (cond_bih + cond_bhh).astype(ml_dtypes.bfloat16).reshape(1, 4 * CH),
        "wpre": _chunk_q8(pre_w, 8, W_SCALE),
        "bpre": _bias_cols(pre_b, 16),
        "wctx": wctx_p,
        "bdec": _bias_cols(dec_bih + dec_bhh + tok_bias, 32),
        "wtok": _chunk_q8(w_tok, 4, W_SCALE),
        "whh": _chunk_q8(dec_Whh, 8, W_SCALE),
        "wout": _chunk_t(out_w, 8),
        "bout": _bias_cols(out_b, 4),
        "tokneg": _bias_cols(-tok_bias, 32),
        "bpre64": _bias_cols(pre_b * np.float32(H_SCALE), 16),
    }


def pack_data(z: np.ndarray, x: np.ndarray, s_steps=S, n_steps=N):
    """Returns per-core zt [128,4,bl] bf16 and xt [n-1,128,4,rows] e4m3.

    Partition dim second so each tensor loads as one contiguous DMA:
    zt[p, kc, b] = z[b, kc*128+p]; xt[j, p, kc, (s, b)] = tok slab j at
    t = kc*128+p (slab j feeds decoder step j+1). Tokens are centered
    (tok - 0.5) and scaled by 64 for the fp8 DoubleRow matmuls.
    """
    b = z.shape[0]
    bl = b // NCORES
    zt = z.T.reshape(4, 128, b).transpose(1, 0, 2).astype(ml_dtypes.bfloat16)
    # x [B, S*N, T] -> [N, T, S, B] -> [N, p, kc, S, B]
    xr = np.ascontiguousarray(
        (x.reshape(b, s_steps, n_steps, T).transpose(2, 3, 1, 0)[: n_steps - 1]
         .astype(np.float32) - 0.5) * np.float32(H_SCALE)
    ).astype(ml_dtypes.float8_e4m3)
    xt = xr.reshape(n_steps - 1, 4, 128, s_steps, b).transpose(0, 2, 1, 3, 4)
    zts, xts = [], []
    for c in range(NCORES):
        sl = slice(c * bl, (c + 1) * bl)
        zts.append(np.ascontiguousarray(zt[:, :, sl]))
        xts.append(
            np.ascontiguousarray(xt[..., sl]).reshape(
                n_steps - 1, 128, 4, s_steps * bl
            )
        )
    return zts, xts


_NC_CACHE = {}


def kernel(z, x, lin_in_w, lin_in_b, cond_Wih, cond_Whh, cond_bih, cond_bhh,
           pre_w, pre_b, dec_Wih, dec_Whh, dec_bih, dec_bhh, out_w, out_b):
    from concourse.bass_utils import run_bass_kernel_spmd

    args = [z, x, lin_in_w, lin_in_b, cond_Wih, cond_Whh, cond_bih, cond_bhh,
            pre_w, pre_b, dec_Wih, dec_Whh, dec_bih, dec_bhh, out_w, out_b]
    (z, x, lin_in_w, lin_in_b, cond_Wih, cond_Whh, cond_bih, cond_bhh,
     pre_w, pre_b, dec_Wih, dec_Whh, dec_bih, dec_bhh, out_w, out_b) = [
        np.asarray(a, dtype=np.float32) for a in args
    ]

    if "nc" not in _NC_CACHE:
        _NC_CACHE["nc"] = build_nc()
    nc = _NC_CACHE["nc"]

    shared = pack_shared(
        lin_in_w, lin_in_b, cond_Whh, cond_bih, cond_bhh,
        pre_w, pre_b, dec_Wih, dec_Whh, dec_bih, dec_bhh, out_w, out_b,
    )
    zts, xts = pack_data(z, x)
    in_maps = [{**shared, "zt": zts[c], "xt": xts[c]} for c in range(NCORES)]

    res = run_bass_kernel_spmd(nc, in_maps, list(range(NCORES)))

    out = np.empty((B, S, N, T), dtype=np.float32)
    for c in range(NCORES):
        # per-core out [N, T, S*bl] -> [bl, S, N, T]
        oc = (res.results[c]["out"].astype(np.float32)
              .reshape(N, T, S, BL).transpose(3, 2, 0, 1))
        out[c * BL : (c + 1) * BL] = oc
    return out.reshape(B, S * N, T)

